# revision 3
# baseline (speedup 1.0000x reference)
"""Gcs pairwise-distance loss kernel for Trainium2 (Bass/Tile), 8-core SPMD.

Math: with d = pred - truth, dX = d[:, :P], dY = d[:, P:] (B=32, P=1024),
    sumsq_h[i] = sum_{b,j} (v[b,j] - v[b,i])^2
               = S2_h + sum_b (1024*v[b,i]^2 - 2*rs_h[b]*v[b,i])
where rs_h[b] = sum_j v[b,j], S2_h = sum_{b,j} v[b,j]^2.  The loss is
    (sum_i sqrt(sumsq_X[i]) + sum_i sqrt(sumsq_Y[i])) / 64.
This collapses the O(B*P^2) pairwise reduction to O(B*P).

Layout: d [32, 2048] is viewed as [128, 512]; partition p = 4*b + c where
c in {0,1} covers X columns and {2,3} covers Y columns.  Per-partition
free-axis reduces give chunk sums; tiny masked matmuls re-associate the
partition-axis sums; a final Sqrt activation with per-row bias and a
4-element dot produce the scalar.

Every core computes the full replicated result (inputs are only 512KB, far
below the ~20us collective all-reduce floor, so replication beats
batch-sharding + AllReduce); core 0's scalar is returned.
"""

import numpy as np

_CACHE = {}


def _build_consts() -> np.ndarray:
    # [128, 137] f32 packed constants (one DMA):
    #   cols 0:4    mask01[p,m]  = 1 if p%4==m          (lhsT, main matmul)
    #   cols 4:8    maskS[p,m]   = 1/1024 if (p%4)//2 == m//2  (lhsT, S2 matmul)
    #   cols 8:136  hconst[k,m]  = -2 if k//2==m//2     (lhsT, pair-sum matmul)
    #   col  136    q4[p]        = 1/64 for p<4         (rhs, final dot)
    c = np.zeros((128, 137), dtype=np.float32)
    p = np.arange(128)
    for m in range(4):
        c[p[p % 4 == m], m] = 1.0
        c[p[(p % 4) // 2 == m // 2], 4 + m] = 1.0 / 1024.0
    k = np.arange(128)
    for m in range(128):
        c[k[k // 2 == m // 2], 8 + m] = -2.0
    c[0:4, 136] = 1.0 / 64.0
    return c


def _build_nc():
    import concourse.tile as tile
    from concourse import bacc, mybir

    f32 = mybir.dt.float32
    nc = bacc.Bacc("TRN2", target_bir_lowering=False, debug=False)
    pred = nc.dram_tensor("pred", [128, 512], f32, kind="ExternalInput").ap()
    truth = nc.dram_tensor("truth", [128, 512], f32, kind="ExternalInput").ap()
    consts = nc.dram_tensor("consts", [128, 137], f32, kind="ExternalInput").ap()
    out = nc.dram_tensor("out", [1, 1], f32, kind="ExternalOutput").ap()

    with tile.TileContext(nc) as tc:
        with (
            tc.tile_pool(name="sb", bufs=1) as sb,
            tc.tile_pool(name="ps", bufs=1, space="PSUM") as ps,
        ):
            tcst = sb.tile([128, 137], f32, tag="tcst")
            nc.sync.dma_start(tcst[:, :], consts)
            tp = sb.tile([128, 512], f32, tag="tp")
            nc.sync.dma_start(tp[:, :], pred)
            tt = sb.tile([128, 512], f32, tag="tt")
            nc.sync.dma_start(tt[:, :], truth)

            mask01 = tcst[:, 0:4]
            maskS = tcst[:, 4:8]
            hconst = tcst[:, 8:136]
            q4 = tcst[0:4, 136:137]

            td = sb.tile([128, 512], f32, tag="td")
            nc.vector.tensor_sub(td[:, :], tp[:, :], tt[:, :])

            # dsq1024 = (32*d)^2 = 1024*d^2 ; cs1024[p] = sum_j 1024*d[p,j]^2
            # (tensor_tensor_reduce crashes TRN2 here; ACT Square+accum works)
            dsq = sb.tile([128, 512], f32, tag="dsq")
            cs1024 = sb.tile([128, 1], f32, tag="cs1024")
            nc.scalar.activation(
                dsq[:, :], td[:, :], mybir.ActivationFunctionType.Square,
                scale=32.0, accum_out=cs1024[:, :],
            )

            # cs_d[p] = sum_j d[p,j]
            cs_d = sb.tile([128, 1], f32, tag="cs_d")
            nc.vector.tensor_reduce(
                out=cs_d[:, :], in_=td[:, :], axis=mybir.AxisListType.X,
                op=mybir.AluOpType.add,
            )

            # hsm2[p] = -2 * (cs_d[p] + cs_d[p^1])  (pair sums via matmul)
            hsm2 = ps.tile([128, 1], f32, tag="hsm2")
            nc.tensor.matmul(hsm2[:, :], hconst, cs_d[:, :], start=True, stop=True)

            # S2 bias per output row m: bias[m] = sum_{p in half(m)} rowsq
            s2 = ps.tile([4, 1], f32, tag="s2")
            nc.tensor.matmul(s2[:, :], maskS, cs1024[:, :], start=True, stop=True)
            bias = sb.tile([4, 1], f32, tag="bias")
            nc.vector.tensor_copy(bias[:, :], s2[:, :])

            # rhs_comb = d*hsm2 + 1024*d^2  (hsm2 staged through SBUF; DVE
            # per-partition scalar reads from PSUM are unproven on this HW)
            hsm2_sb = sb.tile([128, 1], f32, tag="hsm2_sb")
            nc.vector.tensor_copy(hsm2_sb[:, :], hsm2[:, :])
            comb = sb.tile([128, 512], f32, tag="comb")
            nc.vector.scalar_tensor_tensor(
                out=comb[:, :],
                in0=td[:, :],
                scalar=hsm2_sb[:, :],
                in1=dsq[:, :],
                op0=mybir.AluOpType.mult,
                op1=mybir.AluOpType.add,
            )

            # main[m,j] = sum_{p=m mod 4} comb[p,j]
            main = ps.tile([4, 512], f32, tag="main")
            nc.tensor.matmul(main[:, :], mask01, comb[:, :], start=True, stop=True)

            # dist = sqrt(main + bias); dist_sums[m] = sum_j dist[m,j]
            dist = sb.tile([4, 512], f32, tag="dist")
            dsums = sb.tile([4, 1], f32, tag="dsums")
            nc.scalar.activation(
                dist[:, :], main[:, :], mybir.ActivationFunctionType.Sqrt,
                bias=bias[:, :], scale=1.0, accum_out=dsums[:, :],
            )

            # total = sum_m dist_sums[m] / 64
            tot = ps.tile([1, 1], f32, tag="tot")
            nc.tensor.matmul(tot[:, :], dsums[:, :], q4, start=True, stop=True)
            out_sb = sb.tile([1, 1], f32, tag="out_sb")
            nc.vector.tensor_copy(out_sb[:, :], tot[:, :])
            nc.sync.dma_start(out, out_sb[:, :])

    nc.compile()
    return nc


def _get():
    if "nc" not in _CACHE:
        _CACHE["nc"] = _build_nc()
        _CACHE["consts"] = _build_consts()
    return _CACHE["nc"], _CACHE["consts"]


def kernel(pred, truth) -> np.ndarray:
    from concourse.bass_utils import run_bass_kernel_spmd

    nc, consts = _get()
    p = np.ascontiguousarray(np.asarray(pred, dtype=np.float32)).reshape(128, 512)
    t = np.ascontiguousarray(np.asarray(truth, dtype=np.float32)).reshape(128, 512)
    in_map = {"pred": p, "truth": t, "consts": consts}
    res = run_bass_kernel_spmd(
        nc, [dict(in_map) for _ in range(8)], core_ids=list(range(8))
    )
    return res.results[0]["out"].reshape(()).astype(np.float32)
